# revision 2
# baseline (speedup 1.0000x reference)
"""Masked multi-head attention on 8 Trainium2 NeuronCores.

Problem: B=2, H=12, S=2048, D=64 attention with an int32 {0,1} mask
broadcast over heads.  out = softmax(mask ? QK^T/8 : -inf) @ V.

Sharding (8 cores, no cross-core comm):
  core c -> (b = c>>2, head-group hg = (c>>1)&1 -> 6 heads, q-half qh = c&1
  -> 1024 queries).  Each core computes full attention (all 2048 keys) for
  its 6 heads x 1024 queries.

Per-core device algorithm (fp16 matmuls, fp32 accumulation):
  - scoresT[k, q] = K^T @ Q in [k, q] layout with PE row-pair tiling
    (two k-tiles stream concurrently on 64-row PE groups).  K^T is
    pre-scaled on host by 1024*log2(e)/8 so PSUM scores are in
    "fp16-bits log2 domain": z = 1024*log2(e^(s/8)).
  - exp is SPLIT across two engines (it is the serial bottleneck:
    12.6M score elements/core at 1 elem/lane/cycle):
      * S-tiles: ScalarE ACTIVATE Exp with scale=ln2/1024 (exact exp).
      * D-tiles: custom DVE op EXP3_ANT building the fp16 BIT PATTERN of
        2^(z/1024) directly: r = RNE_1024(z) via the fp32 magic-add trick,
        g = z - r in [-512,512), bits = (c2*g + c1)*g + c0 + r (minimax
        quadratic mantissa fit, max err ~0.5%), written through a
        saturating fp32->uint16 convert into an fp16-viewed tile.
    This runs exp on ScalarE and VectorE concurrently (~1.35x throughput).
  - mask: probs *= maskT tile (fp16 {0,1}); per-tile routed to VectorE or
    the otherwise-idle GpSimd (Pool) engine to offload DVE.
  - AV with V stationary: lhsT = [V_ktile | ones] (65 cols), rhs = streamed
    probsT [128k, 512q] -> out[d, q] accumulates over the 16 k-tiles in two
    single-bank PSUM accumulators; column 64 accumulates the softmax
    denominator for free.
  - AV for score-tile i is emitted after exp of tile i+AVLAG so the in-order
    PE queue never blocks on a mask-DMA-gated tile.

Host does dtype/layout prep (fp16 conversion, pair-stacked scaled K^T,
V|ones, mask^T as fp16 {0,1}) and the final divide-by-denominator +
[d,q]->[q,d] transpose.

PSUM budget (8 banks): scores [128,2048]+[128,1024] alternating = 6, AV
accumulators 2x[65,512] = 2.

Scheduling notes (hardware-measured): DMA issue instructions serialize at
~0.6us each on the sync engine after a ~7us runtime preamble, so emission
order is arrival priority; output DMAs are emitted last since a pending DMA
issue blocks its engine.  Denser/full-width DMA bursts trip the chip's power
limiter (throttle_activity_1) and slow EVERY engine ~20% -- the narrow
two-half Q loads and 16 small mask DMAs here stay under it.
"""

import os
import sys

import numpy as np

for _p in ("/opt/trn_rl_repo",):
    if _p not in sys.path and os.path.isdir(_p):
        sys.path.insert(0, _p)

import concourse.bass as bass
import concourse.mybir as mybir
import concourse.tile as tile
from concourse import bacc
from concourse.bass_utils import run_bass_kernel_spmd
import concourse.dve_ops as dve_ops
from concourse.dve_ops import DveOp
from concourse.dve_spec import Spec, Src0, C0, C1, C2, C3, lower, _spill_c3_to_src1
from concourse.dve_uop import DveOpSpec

FP16 = mybir.dt.float16
F32 = mybir.dt.float32
U16 = mybir.dt.uint16

B, H, S, D = 2, 12, 2048, 64
NCORES = 8
HPC = 6        # heads per core
QPC = 1024     # queries per core
KT = S // 128  # 16 k-tiles

# log2-domain pre-scale folded into K^T on host: scores land in PSUM as
# z = 1024*log2(e) * (QK/8); exp(s/8) = 2^(z/1024).
LOG2E = 1.4426950408889634
KSCALE = 1024.0 * LOG2E * 0.125
ACT_SCALE = float(np.log(2) / 1024.0)  # ScalarE: exp(ACT_SCALE * z) = e^(s/8)

# EXP3_ANT constants: bits = (C2QUAD*g + C1LIN)*g + C0FIT + r
MAGIC = 1.5 * 2**33          # fp32 ulp=1024 -> r = RNE_1024(z)
C0FIT = 425.106097           # minimax quadratic mantissa fit on [-512,512)
C1LIN = 0.988969081
C2QUAD = 3.257847e-04

# Per-head score tiles, in 512-column chunks (chunk c -> k-tile c//2,
# q-half c%2).  B tiles = 1024 cols (2 chunks, 2 PSUM banks), A tiles =
# 2048 cols (4 chunks, 4 banks).  Alternating B,A keeps two score tiles in
# flight inside 6 PSUM banks (the other 2 hold the AV accumulators).
SEQ = ((2, "b"), (4, "a")) * 5 + ((2, "b"),)
assert sum(n for n, _ in SEQ) == 2 * KT
AVLAG = 2  # score tiles between exp and AV consumption

# Engine routing, tile index 0..10 within each head (tunable):
#   ROUTE_D: tiles whose exp runs on the DVE custom op (rest: ScalarE).
#   POOL_MASK: tiles whose mask multiply runs on GpSimd (rest: VectorE).
ROUTE_D = frozenset(int(x) for x in
                    os.environ.get("ATT_ROUTE_D", "3,8").split(",") if x != "")
POOL_MASK = frozenset(int(x) for x in
                      os.environ.get("ATT_POOL_MASK", "1,5,9").split(",") if x != "")

_NC_CACHE = None
_EXP3_OP = None


def get_exp3_op():
    """Register (once) the custom DVE op computing fp16 exp bits.

    body (8 ALU stages, fp32 pipeline):
      t = z + MAGIC ; r = t - MAGIC          # r = RNE_1024(z), exact
      g = z - r                              # g in [-512, 512)
      bits = ((C3*g + C2)*g + C1) + r        # C3 spilled to in1 ([P,1] AP)
    out dtype uint16: saturating RNE fp32->uint16 convert writes the fp16
    bit pattern of ~2^(z/1024) (up to a uniform scale, which softmax
    normalization cancels).
    """
    global _EXP3_OP
    if _EXP3_OP is not None:
        return _EXP3_OP
    name = "EXP3_ANT"
    for op in dve_ops.OPS:
        if op.name == name:
            _EXP3_OP = op
            return op
    t = Src0 + C0
    r = t - C0
    g = Src0 - r
    body = _spill_c3_to_src1(((C3 * g + C2) * g + C1) + r)
    spec = Spec(body=body)
    row = dve_ops._CUSTOM_DVE_ROW_BASE + len(dve_ops.OPS)
    dve_ops._SUB_OPCODE_FOR_NAME[name] = row
    shas = {}
    for ver in ("v3", "v4"):
        s = DveOpSpec(name=name, opcode=row, uops=lower(spec, ver=ver), rd1_en=True)
        shas[ver] = s.sha(ver)
    op = DveOp(name, spec, subdim=False, uops_sha=shas)
    dve_ops.OPS.append(op)
    dve_ops.CUSTOM_DVE_SPECS[name] = spec
    _EXP3_OP = op
    return op


def build_bass():
    """Build the single-core Bass/Tile program (SPMD across 8 cores)."""
    exp3 = get_exp3_op()
    nc = bacc.Bacc("TRN2", target_bir_lowering=False, debug=False)

    qt = nc.declare_dram_parameter("qt", [HPC, 128, QPC], FP16, isOutput=False)
    kt = nc.declare_dram_parameter("kt", [HPC, 128, QPC], FP16, isOutput=False)
    vt = nc.declare_dram_parameter("vt", [HPC, 128, KT, 65], FP16, isOutput=False)
    mt = nc.declare_dram_parameter("mt", [KT, 128, QPC], FP16, isOutput=False)
    o = nc.declare_dram_parameter("o", [HPC, 65, QPC], F32, isOutput=True)

    with tile.TileContext(nc) as tc:
        with (
            tc.tile_pool(name="const", bufs=1) as const,
            tc.tile_pool(name="prA", bufs=6) as prA_pool,
            tc.tile_pool(name="prB", bufs=8) as prB_pool,
            tc.tile_pool(name="outp", bufs=2) as outp,
            tc.tile_pool(name="pa", bufs=1, space="PSUM") as pa,
            tc.tile_pool(name="pb", bufs=1, space="PSUM") as pb,
            tc.tile_pool(name="pv0", bufs=1, space="PSUM") as pv0,
            tc.tile_pool(name="pv1", bufs=1, space="PSUM") as pv1,
        ):
            # Resident fp16 operands (loaded straight from DRAM, no casts).
            qh = const.tile([128, HPC, QPC], FP16)
            kh = const.tile([128, HPC, QPC], FP16)
            vh = const.tile([128, HPC, KT, 65], FP16)
            mk = [
                const.tile([128, QPC], FP16, name=f"mk{t}", tag=f"mk{t}")
                for t in range(KT)
            ]
            # [128,1] broadcast carrying the spilled C3 (quadratic coeff).
            c3t = const.tile([128, 1], F32, name="c3t")
            nc.vector.memset(c3t[:], C2QUAD)

            def load_head(h):
                nc.sync.dma_start(kh[:, h, :], kt[h])
                nc.sync.dma_start(qh[0:64, h, :], qt[h][0:64, :])
                nc.sync.dma_start(qh[64:128, h, :], qt[h][64:128, :])
                nc.sync.dma_start(vh[:, h, :, :], vt[h])

            load_head(0)
            for t in range(KT):
                nc.sync.dma_start(mk[t][:], mt[t])
            for h in range(1, HPC):
                load_head(h)

            avs = [None, None]  # per-q-half AV accumulators for current head

            def emit_av(ent):
                """AV matmuls (and head epilogue) for a finished score tile."""
                h, pr, c0, n = ent
                for ci in range(n):
                    c = c0 + ci
                    t, qc = c // 2, c % 2
                    if t == 0:
                        pool = pv0 if qc == 0 else pv1
                        avs[qc] = pool.tile(
                            [65, 512], F32, name=f"av{qc}", tag="av"
                        )
                    nc.tensor.matmul(
                        avs[qc][:],
                        vh[:, h, t, :],
                        pr[:, 512 * ci : 512 * (ci + 1)],
                        start=(t == 0),
                        stop=(t == KT - 1),
                    )
                if c0 + n == 2 * KT:
                    osb = outp.tile([65, QPC], F32, tag="os")
                    if h == HPC - 1:
                        # Last head: run the two PSUM evacuations in
                        # parallel on scalar+vector to trim the kernel tail.
                        nc.scalar.copy(osb[:, 0:512], avs[0][:])
                    else:
                        nc.vector.tensor_copy(osb[:, 0:512], avs[0][:])
                    nc.vector.tensor_copy(osb[:, 512:QPC], avs[1][:])
                    nc.sync.dma_start(o[h], osb[:])

            pending = []
            for h in range(HPC):
                c0 = 0
                for ti, (n, kind) in enumerate(SEQ):
                    w = 512 * n
                    if kind == "a":
                        sc = pa.tile([128, w], F32, tag="sa")
                        pr = prA_pool.tile([128, w], FP16, tag="pra")
                    else:
                        sc = pb.tile([128, w], F32, tag="sb")
                        pr = prB_pool.tile([128, w], FP16, tag="prb")
                    for ci in range(n):
                        c = c0 + ci
                        t, qc = c // 2, c % 2
                        r, a = t % 2, t // 2
                        nc.tensor.matmul(
                            sc[:, 512 * ci : 512 * (ci + 1)],
                            kh[64 * r : 64 * r + 64, h, 128 * a : 128 * a + 128],
                            qh[64 * r : 64 * r + 64, h, 512 * qc : 512 * (qc + 1)],
                            start=True,
                            stop=True,
                            tile_position=(64 * r, 0),
                        )
                    if len(pending) == AVLAG:
                        emit_av(pending.pop(0))
                    if ti in ROUTE_D:
                        # fp16 exp bits via DVE (frees ScalarE).
                        nc.vector._custom_dve(
                            exp3,
                            out=pr[:].bitcast(U16),
                            in0=sc[:],
                            in1=c3t[:],
                            s0=MAGIC,
                            s1=C0FIT,
                            imm2=C1LIN,
                        )
                    else:
                        nc.scalar.activation(
                            pr[:],
                            sc[:],
                            mybir.ActivationFunctionType.Exp,
                            scale=ACT_SCALE,
                        )
                    # Mask multiplies: one tensor_mul per (k-tile, covered
                    # q-half run) of this tile; VectorE or GpSimd per route.
                    eng = nc.gpsimd if ti in POOL_MASK else nc.vector
                    ci = 0
                    while ci < n:
                        c = c0 + ci
                        t, qc = c // 2, c % 2
                        m = 2 if (qc == 0 and ci + 1 < n) else 1
                        eng.tensor_mul(
                            pr[:, 512 * ci : 512 * (ci + m)],
                            pr[:, 512 * ci : 512 * (ci + m)],
                            mk[t][:, 512 * qc : 512 * (qc + m)],
                        )
                        ci += m
                    pending.append((h, pr, c0, n))
                    c0 += n
            for ent in pending:
                emit_av(ent)

    nc.compile()
    return nc


def _shard(c, Q, K, V, mask):
    b, hg, qhf = c >> 2, (c >> 1) & 1, c & 1
    hs = slice(hg * HPC, hg * HPC + HPC)
    qs = slice(qhf * QPC, qhf * QPC + QPC)
    # qt[h, 64r+d, q] = Q[b, h, qs+q, d] (duplicated on both PE row groups)
    qq = Q[b, hs, qs, :].transpose(0, 2, 1).astype(np.float16)
    qtv = np.ascontiguousarray(np.concatenate([qq, qq], axis=1))
    # kt[h, 64r+d, 128a+cc] = KSCALE * K[b, h, 256a+128r+cc, d] (pair-stacked)
    kk = (K[b, hs, :, :] * KSCALE).reshape(HPC, KT // 2, 2, 128, 64)
    kk = kk.transpose(0, 2, 4, 1, 3)
    ktv = np.ascontiguousarray(kk).reshape(HPC, 128, QPC).astype(np.float16)
    # vt[h, p, t, 0:64] = V[b, h, 128t+p, :], col 64 = 1.0
    vtv = np.ones((HPC, 128, KT, 65), np.float16)
    vtv[..., 0:64] = V[b, hs, :, :].reshape(HPC, KT, 128, 64).transpose(0, 2, 1, 3)
    # mt[t, p, q] = mask[b, 0, qs+q, 128t+p]
    mtv = mask[b, 0, qs, :].T.reshape(KT, 128, QPC).astype(np.float16)
    return {"qt": qtv, "kt": ktv, "vt": vtv, "mt": mtv}


def get_nc():
    global _NC_CACHE
    if _NC_CACHE is None:
        _NC_CACHE = build_bass()
    return _NC_CACHE


def kernel(Q, K, V, mask):
    Q = np.asarray(Q, dtype=np.float32)
    K = np.asarray(K, dtype=np.float32)
    V = np.asarray(V, dtype=np.float32)
    mask = np.asarray(mask, dtype=np.int32)

    in_maps = [_shard(c, Q, K, V, mask) for c in range(NCORES)]
    res = run_bass_kernel_spmd(get_nc(), in_maps, list(range(NCORES))).results

    out = np.empty((B, H, S, D), dtype=np.float32)
    for c in range(NCORES):
        b, hg, qhf = c >> 2, (c >> 1) & 1, c & 1
        oc = res[c]["o"]  # [HPC, 65, QPC]: rows 0-63 = V-weighted sums, 64 = denom
        blk = (oc[:, 0:64, :] / oc[:, 64:65, :]).transpose(0, 2, 1)
        out[b, hg * HPC : hg * HPC + HPC, qhf * QPC : qhf * QPC + QPC, :] = blk
    return out
